# revision 12
# baseline (speedup 1.0000x reference)
"""Correntropy loss on 8 Trainium2 NeuronCores — fp16-staged, TensorE gram.

Reference math (all f32):
    t = (target - 0.5) * 2 ; o = (output - 0.5) * 2
    cost = mean(1 - exp(-sigma * (o - t)^2)),  sigma = 1/1000
Since o - t == 2*(output - target), this equals
    mean(1 - exp(-c * w)),  w = (output - target)^2,  c = 4*sigma = 0.004

The kernel is pure HBM-bandwidth bound (512 MiB of f32 inputs).  The
host stages the device buffers in float16, halving HBM traffic (fp16
round-to-nearest bias on S1 is ~1e-7 relative — far below the 2e-2
tolerance).  Device computes power sums of w; host evaluates the 1-exp
Taylor series in f64:  sum(1-exp(-c*w)) = c*S1 - c^2/2*S2 + O(c^3*S3).
S1 is exact over all elements; S2 (a 3.2e-3 relative correction) is
computed on 2 of 16 tiles and scaled by 8; the dropped S3 term is
+9.1e-6 relative.

Engine layout (per core, 16 tiles of [128 x 4000] per operand, fp16).
The DMA stream (~33 MB ≈ 77 us) must stay the critical path, and an
earlier revision showed that chaining DMA->DVE->TensorE per tile lets
the tile scheduler couple the engines into a lockstep that stalls the
stream.  So the heavy tiles skip DVE entirely: their columns are
host-packed as 63 chunks of [o(64)|t(64)] (zero-padded) and TensorE
runs a self-gram matmul(C += blk.T @ blk) for each 128-column block,
accumulating one [128,128] PSUM bank.  Then
    S1 = sum_k C[k,k] + C[64+k,64+k] - 2*C[k,64+k]   (done on host)
since sum (o-t)^2 = sum o^2 - 2*sum o*t + sum t^2.
  gram tiles {0,2,4,5,7,8,10,12}: DMA -> 63x matmul   (PE only)
  ACT tiles {1,3,6,9,11,13}:      DVE sub; ACT Square(d) accum S1;
                                  tiles {3,9} also Square(w) accum S2
  tail tiles {14,15}:             DVE sub; ACT Square accum S1, in
              tapered column slices so the post-stream chain is ~1 us
Worst-case engine busy (PE fully cold at 1.2 GHz): PE ~54 us, ACT ~40,
DVE ~17 — all far below the DMA window, so no backlog can build up.
The PSUM gram is copied to SBUF and DMA'd out mid-stream; the host
reduces everything in f64 and applies the series (the scalar
"all-reduce" of the sharding hint, done exactly on the host).

DRAM layout: one flat fp16 array of per-piece blocks [128, width].
"""

import numpy as np

import concourse.bacc as bacc
import concourse.mybir as mybir
import concourse.tile as tile
from concourse.bass_utils import run_bass_kernel_spmd

N_CORES = 8
ROWS = 65536
COLS = 1000
ROWS_PER_CORE = ROWS // N_CORES  # 8192
P = 128  # SBUF partitions

Q = 4  # rows folded into the free dim per partition
FREE = Q * COLS  # 4000 elements of one operand per partition per tile
N_TILES = ROWS_PER_CORE // (P * Q)  # 16

CH = 64  # gram chunk width per operand
N_CHUNK = -(-FREE // CH)  # 63 chunks (last one zero-padded)
GFREE = N_CHUNK * CH  # 4032 padded columns per operand
BLK = 2 * CH  # 128-wide [o64|t64] gram block

# Tile kind sequence in stream order.  Gram tiles are spread through
# the first 12 positions (the last gram lands early enough that the PE
# finishes well before the stream ends); the two S2 tiles sit early,
# where ACT is otherwise idle, so their double squares are absorbed.
_KINDS = ["g", "s2", "g", "g", "s2", "g", "g", "a",
          "g", "g", "a", "g", "a", "a", "t", "t"]
GRAM_TILES = tuple(i for i, k in enumerate(_KINDS) if k == "g")
S2_TILES = tuple(i for i, k in enumerate(_KINDS) if k == "s2")
S2_SCALE = float(N_TILES) / len(S2_TILES)  # 8.0

# Tail taper: the last two tiles are split into smaller column pieces so
# the serial sub->square chain after the final DMA is short.
_WIDTHS = [1400, 1200, 800, 400, 200]
_OFFS = [sum(_WIDTHS[:_k]) for _k in range(len(_WIDTHS))]
_SLICES = {N_TILES - 2: [(0, 2000), (2000, 2000)],
           N_TILES - 1: list(zip(_OFFS, _WIDTHS))}

# Pieces: (tile, kind, off, z).  Gram tiles move 2*GFREE interleaved
# elems; ACT/tail pieces move [o(z)|t(z)] halves.
PIECES = []
for _t in range(N_TILES):
    if _t in GRAM_TILES:
        PIECES.append((_t, "gram", 0, GFREE))
    else:
        for _off, _z in _SLICES.get(_t, [(0, FREE)]):
            PIECES.append((_t, "act", _off, _z))
N_PIECES = len(PIECES)  # 8 gram + 6 full act + 7 taper = 21
S1_PIECE_IDX = [i for i, p in enumerate(PIECES) if p[1] == "act"]
S2_PIECE_IDX = [i for i, p in enumerate(PIECES) if p[0] in S2_TILES]
ACC_COLS = 2 * N_PIECES
TOTAL_ELEMS = sum(P * 2 * p[3] for p in PIECES)
N_GRAM_MM = len(GRAM_TILES) * N_CHUNK  # 504

F32 = mybir.dt.float32
F16 = mybir.dt.float16


def _build():
    nc = bacc.Bacc()
    comb_p = nc.declare_dram_parameter("combined", [TOTAL_ELEMS], F16, isOutput=False)
    acc_p = nc.declare_dram_parameter("partial", [P, ACC_COLS], F32, isOutput=True)
    gram_p = nc.declare_dram_parameter("gram", [BLK, BLK], F32, isOutput=True)

    with tile.TileContext(nc) as tc:
        with (
            tc.tile_pool(name="io", bufs=6) as io_pool,
            tc.tile_pool(name="work", bufs=1) as work_pool,
            tc.tile_pool(name="accp", bufs=1) as acc_pool,
            tc.psum_pool(name="gr", bufs=1) as psum_pool,
        ):
            acc = acc_pool.tile([P, ACC_COLS], F32)
            gram = psum_pool.tile([BLK, BLK], F32)
            gram_sb = acc_pool.tile([BLK, BLK], F32)
            mm_idx = 0
            ofs = 0
            for i, (t, kind, off, z) in enumerate(PIECES):
                # Separate rotations for gram vs ACT pieces: a lagging PE
                # then only gates gram DMAs, never the tail ACT pieces.
                if kind == "gram":
                    ab = io_pool.tile([P, 2 * z], F16, tag="abg", bufs=5)
                else:
                    ab = io_pool.tile([P, 2 * z], F16, tag="aba", bufs=4)
                nc.sync.dma_start(
                    out=ab[:],
                    in_=comb_p[ofs : ofs + P * 2 * z].rearrange("(p m) -> p m", p=P),
                )
                ofs += P * 2 * z
                if kind == "gram":
                    for b in range(N_CHUNK):
                        blk = ab[:, b * BLK : (b + 1) * BLK]
                        nc.tensor.matmul(
                            gram[:],
                            blk,
                            blk,
                            start=(mm_idx == 0),
                            stop=(mm_idx == N_GRAM_MM - 1),
                        )
                        mm_idx += 1
                    if mm_idx == N_GRAM_MM:
                        nc.scalar.copy(gram_sb[:], gram[:])
                        nc.sync.dma_start(out=gram_p[:], in_=gram_sb[:])
                else:
                    d = work_pool.tile([P, z], F16, tag="d", bufs=2)
                    nc.vector.tensor_sub(d[:], ab[:, 0:z], ab[:, z : 2 * z])
                    w = work_pool.tile([P, z], F16, tag="w", bufs=2)
                    nc.scalar.activation(
                        w[:],
                        d[:],
                        mybir.ActivationFunctionType.Square,
                        accum_out=acc[:, i : i + 1],
                    )
                    if t in S2_TILES:
                        w2 = work_pool.tile([P, z], F16, tag="w2", bufs=1)
                        nc.scalar.activation(
                            w2[:],
                            w[:],
                            mybir.ActivationFunctionType.Square,
                            accum_out=acc[:, N_PIECES + i : N_PIECES + i + 1],
                        )
            assert mm_idx == N_GRAM_MM
            nc.sync.dma_start(out=acc_p[:], in_=acc[:])
    nc.finalize()
    return nc


_NC = None


def _get_nc():
    global _NC
    if _NC is None:
        _NC = _build()
    return _NC


def _pack_gram_tile(o_t, t_t):
    """[P, FREE] x2 -> [P, 2*GFREE] as 63 chunks of [o(64)|t(64)]."""
    pad = GFREE - FREE
    o_p = np.pad(o_t, ((0, 0), (0, pad)))
    t_p = np.pad(t_t, ((0, 0), (0, pad)))
    o_c = o_p.reshape(P, N_CHUNK, CH)
    t_c = t_p.reshape(P, N_CHUNK, CH)
    return np.stack([o_c, t_c], axis=2).reshape(P, 2 * GFREE)


def _shard_inputs(output, target):
    output = np.asarray(output)
    target = np.asarray(target)
    in_maps = []
    for ci in range(N_CORES):
        sl = slice(ci * ROWS_PER_CORE, (ci + 1) * ROWS_PER_CORE)
        o4 = output[sl].astype(np.float16).reshape(N_TILES, P, FREE)
        t4 = target[sl].astype(np.float16).reshape(N_TILES, P, FREE)
        blocks = []
        for t, kind, off, z in PIECES:
            if kind == "gram":
                blk = _pack_gram_tile(o4[t], t4[t])
            else:
                blk = np.concatenate(
                    [o4[t, :, off : off + z], t4[t, :, off : off + z]], axis=1
                )
            blocks.append(blk.reshape(-1))
        comb = np.concatenate(blocks)
        assert comb.size == TOTAL_ELEMS
        in_maps.append({"combined": comb})
    return in_maps


def run_device(output, target, trace=False):
    """Returns (per-core (partial, gram) pairs, BassKernelResults)."""
    in_maps = _shard_inputs(output, target)
    res = run_bass_kernel_spmd(_get_nc(), in_maps, list(range(N_CORES)), trace=trace)
    partials = [
        (res.results[i]["partial"], res.results[i]["gram"]) for i in range(N_CORES)
    ]
    return partials, res


def _reduce(partials):
    s1 = s2 = 0.0
    for p, g in partials:
        p64 = p.astype(np.float64)
        g64 = g.astype(np.float64)
        dg = np.diag(g64)
        s1 += dg[:CH].sum() + dg[CH:].sum() - 2.0 * np.diag(g64[:CH, CH:]).sum()
        for i in S1_PIECE_IDX:
            s1 += p64[:, i].sum()
        for i in S2_PIECE_IDX:
            s2 += p64[:, N_PIECES + i].sum()
    s2 *= S2_SCALE
    c = 4.0 * float(np.float32(1.0 / COLS))  # match reference's f32 sigma
    total = c * s1 - (c * c / 2.0) * s2
    n = float(ROWS) * float(COLS)
    return np.array(total / n, dtype=np.float32)


def kernel(output, target):
    partials, _ = run_device(output, target)
    return _reduce(partials)
